# revision 14
# baseline (speedup 1.0000x reference)
"""Trainium2 Bass kernel for a causal multi-head attention block.

Reference computation (per nn_Attend):
    q = (x @ Wq + bq), k = (x @ Wk + bk), v = (x @ Wv + bv)   (per-head split)
    att = softmax(causal(q k^T / sqrt(hd)))
    y = (att v) @ Wo + bo

Sharding: tensor-parallel over heads across 8 NeuronCores. Core c gets
heads [2c, 2c+1]: column shards of Wq/Wk/Wv (+bias shards) and the matching
row shard of Wo. Every core computes a full-size partial output
yt_c = (att_out_c @ Wo_c)^T; the host sums the 8 partials, adds bo, and
transposes back.

On-chip layout is "transposed": activations live as [feature, token] so that
every matmul's contraction dim sits on SBUF partitions:
  QT/KT/VT = W^T @ x^T  (lhsT = W in natural [in,out] layout, rhs = x^T)
  scoresT[k, q] = (K^T)^T-slice @ Q^T      (contraction over head_dim)
  att_outT[hd, q] = sum_kt V[kt]^T-as-lhsT @ expT[kt]  (contraction over keys)
  yT[out, q] = Wo-slice-as-lhsT @ att_outT (contraction over per-core head dims)
V is needed in natural [token, hd] layout as lhsT; it is produced by PE
transposes of VT with an extra all-ones column so that each att_outT
accumulation also yields the softmax row-sums for free. Softmax is unmasked
exp (scores are bounded, no max subtraction needed); the causal mask is
"compute only the valid triangle" + a 0/1 upper-triangular mask multiply on
diagonal 128x128 blocks. Normalization divides att_outT columns by the
broadcast row-sums before the output projection.

All matmuls run in float16 (full PE rate, fast weight load).
"""

import os
from contextlib import ExitStack
from dataclasses import dataclass

import numpy as np

import concourse.bass as bass
import concourse.tile as tile
from concourse import bacc, mybir
from concourse.masks import make_identity, make_upper_triangular

F32 = mybir.dt.float32
F32R = mybir.dt.float32r
F16 = mybir.dt.float16
P = 128  # SBUF partitions


@dataclass(frozen=True)
class Cfg:
    B: int = 2
    S: int = 2048
    D: int = 1024
    H: int = 16
    NCORES: int = 8
    RC: int = 512        # row chunk for QKV projection streaming
    PW: int = 1024       # psum piece width for scoresT (2 banks)

    @property
    def HPC(self):  # heads per core
        return self.H // self.NCORES

    @property
    def hd(self):  # head dim
        return self.D // self.H

    @property
    def COLS(self):  # per-core projection output columns
        return self.HPC * self.hd

    @property
    def KT(self):  # contraction tiles for projections
        return self.D // P

    @property
    def ROWS(self):
        return self.B * self.S

    @property
    def SKT(self):  # key tiles per batch
        return self.S // P

    @property
    def QCW(self):  # q chunk width for att_out accumulation
        return min(512, self.S)

    @property
    def NQC(self):
        return self.S // self.QCW

    @property
    def KPC(self):  # key tiles per q-chunk
        return self.QCW // P

    @property
    def VTC(self):  # v-tile columns per head (head_dim + ones column)
        return self.hd + 1


def _exp_offsets(cfg: Cfg):
    """Free-dim offsets of each key-tile's strip in the exp buffer."""
    offs, total = [], 0
    for kt in range(cfg.SKT):
        offs.append(total)
        total += cfg.S - P * kt
    return offs, total


def emit_attention(tc: tile.TileContext, io: dict, cfg: Cfg):
    nc = tc.nc
    xt, wq, wk, wv, wo = io["xt"], io["wq"], io["wk"], io["wv"], io["wo"]
    bq, bk, bv, yt = io["bq"], io["bk"], io["bv"], io["yt"]

    COLS, KT, RC, ROWS = cfg.COLS, cfg.KT, cfg.RC, cfg.ROWS
    NRC = ROWS // RC
    S, SKT, B, hd, HPC = cfg.S, cfg.SKT, cfg.B, cfg.hd, cfg.HPC
    QCW, NQC, KPC, VTC, PW = cfg.QCW, cfg.NQC, cfg.KPC, cfg.VTC, cfg.PW
    NT = ROWS // P  # v row-tiles
    offs, expw = _exp_offsets(cfg)

    with ExitStack() as ctx:
        wpool = ctx.enter_context(tc.tile_pool(name="wpool", bufs=1))
        qkpool = ctx.enter_context(tc.tile_pool(name="qkpool", bufs=1))
        vpool = ctx.enter_context(tc.tile_pool(name="vpool", bufs=1))

        # ---- constants ----
        wq_sb = wpool.tile([P, KT, COLS], F16, tag="wq")
        wk_sb = wpool.tile([P, KT, COLS], F16, tag="wk")
        wv_sb = wpool.tile([P, KT, COLS], F16, tag="wv")
        wo_sb = wpool.tile([COLS, cfg.D], F16, tag="wo")
        bq_sb = wpool.tile([COLS, 1], F32, tag="bq")
        bk_sb = wpool.tile([COLS, 1], F32, tag="bk")
        bv_sb = wpool.tile([COLS, 1], F32, tag="bv")
        ident = wpool.tile([P, P], F16, tag="ident")
        umask = wpool.tile([P, P], F16, tag="umask")

        for w_dram, w_sb in ((wq, wq_sb), (wk, wk_sb), (wv, wv_sb)):
            nc.sync.dma_start(
                out=w_sb, in_=w_dram.rearrange("(kt p) c -> p kt c", p=P)
            )
        nc.sync.dma_start(out=wo_sb, in_=wo)
        for b_dram, b_sb in ((bq, bq_sb), (bk, bk_sb), (bv, bv_sb)):
            nc.sync.dma_start(out=b_sb, in_=b_dram.rearrange("(c one) -> c one", one=1))
        make_identity(nc, ident)
        make_upper_triangular(nc, umask, val=1.0, diag=True)

        # ---- phase A: projections QT/KT/VT + V transpose ----
        qt_sb = qkpool.tile([COLS, ROWS], F16, tag="qt")
        kt_sb = qkpool.tile([COLS, ROWS], F16, tag="kt")
        vt_sb = qkpool.tile([COLS, ROWS], F16, tag="vt")
        v_sb = vpool.tile([P, NT, HPC * VTC], F16, tag="v")
        # ones columns for the row-sum trick
        nc.vector.memset(v_sb[:, :, hd :: VTC], 1.0)

        HC = COLS // 2  # column-tile half

        with (
            tc.tile_pool(name="xpool", bufs=2) as xpool,
            tc.tile_pool(name="psA", bufs=2, space="PSUM") as psA,
            tc.tile_pool(name="psT", bufs=2, space="PSUM") as psT,
        ):
            for rc in range(NRC):
                xt_t = xpool.tile([P, KT, RC], F16, tag="xt")
                nc.sync.dma_start(
                    out=xt_t,
                    in_=xt[:, rc * RC : (rc + 1) * RC].rearrange(
                        "(kt p) n -> p kt n", p=P
                    ),
                )
                for wi, (w_sb, b_sb, dest) in enumerate((
                    (wq_sb, bq_sb, qt_sb),
                    (wk_sb, bk_sb, kt_sb),
                    (wv_sb, bv_sb, vt_sb),
                )):
                    ps = psA.tile([COLS, RC], F32, tag="proj", name=f"proj{rc}_{wi}")
                    # two column-tile chains (T0/T1) run concurrently on PE
                    for kt in range(KT):
                        for half in range(2):
                            nc.tensor.matmul(
                                ps[half * HC : (half + 1) * HC, :],
                                w_sb[:, kt, half * HC : (half + 1) * HC],
                                xt_t[:, kt, :],
                                start=(kt == 0),
                                stop=(kt == KT - 1),
                                skip_group_check=True,
                            )
                    out_ap = dest[:, rc * RC : (rc + 1) * RC]
                    nc.vector.tensor_scalar_add(out_ap, ps, b_sb)
            # transpose VT into natural-layout v tiles (one transpose-mode region)
            for t in range(NT):
                pst = psT.tile([P, COLS], F16, tag="vtr", name=f"vtr{t}")
                nc.tensor.matmul(
                    pst,
                    vt_sb[:, t * P : (t + 1) * P],
                    ident[:COLS, :COLS],
                    is_transpose=True,
                )
                nc.vector.tensor_copy(
                    out=v_sb[:, t, :].rearrange("p (h c) -> p h c", h=HPC)[
                        :, :, 0:hd
                    ],
                    in_=pst.rearrange("p (h c) -> p h c", h=HPC),
                )

        # ---- phases B/C/D per batch, heads interleaved ----
        with (
            tc.tile_pool(name="expool", bufs=1) as expool,
            tc.tile_pool(name="apool", bufs=2) as apool,
            tc.tile_pool(name="spool", bufs=4) as spool,
            tc.tile_pool(name="opool", bufs=3) as opool,
            tc.tile_pool(name="psB", bufs=1, space="PSUM") as psB,
            tc.tile_pool(name="psC", bufs=2, space="PSUM") as psC,
            tc.tile_pool(name="psD", bufs=2, space="PSUM") as psD,
        ):
            for b in range(B):
                att_sb = apool.tile([COLS, S], F16, tag="att", name=f"att{b}")
                exps = [
                    expool.tile([P, expw], F16, tag=f"exp{h}", name=f"exp{b}_{h}")
                    for h in range(HPC)
                ]

                # -- B: scoresT -> exp, both heads' row-tiles in flight --
                for kt in range(SKT):
                    w = S - P * kt
                    off = offs[kt]
                    q0 = b * S + P * kt  # global q col of strip start
                    for p0 in range(0, w, PW):
                        pw_ = min(PW, w - p0)
                        pieces = [
                            psB.tile([P, PW], F32, tag=f"sc{h}",
                                     name=f"sc{b}_{kt}_{p0}_{h}")
                            for h in range(HPC)
                        ]
                        for s0 in range(0, pw_, 512):
                            sw = min(512, pw_ - s0)
                            for h in range(HPC):
                                hp = h * hd
                                nc.tensor.matmul(
                                    pieces[h][:, s0 : s0 + sw],
                                    kt_sb[hp : hp + hd,
                                          b * S + P * kt : b * S + P * (kt + 1)],
                                    qt_sb[hp : hp + hd,
                                          q0 + p0 + s0 : q0 + p0 + s0 + sw],
                                    start=True,
                                    stop=True,
                                )
                        for h in range(HPC):
                            nc.scalar.activation(
                                out=exps[h][:, off + p0 : off + p0 + pw_],
                                in_=pieces[h][:, 0:pw_],
                                func=mybir.ActivationFunctionType.Exp,
                            )
                    for h in range(HPC):
                        # causal mask on the diagonal block: keep q >= k
                        nc.gpsimd.affine_select(
                            out=exps[h][:, off : off + P],
                            in_=exps[h][:, off : off + P],
                            compare_op=mybir.AluOpType.is_ge,
                            fill=0.0,
                            base=0,
                            pattern=[[1, P]],
                            channel_multiplier=-1,
                        )

                # -- C: att_outT accumulation + normalize, heads interleaved --
                for c in range(NQC):
                    kt_hi = min(SKT, KPC * (c + 1))
                    pas = [
                        psC.tile([VTC, QCW], F32, tag="acc", name=f"acc{b}_{c}_{h}")
                        for h in range(HPC)
                    ]
                    for kt in range(kt_hi):
                        lo = max(QCW * c, P * kt)
                        n = QCW * (c + 1) - lo
                        if n <= 0:
                            continue
                        for h in range(HPC):
                            nc.tensor.matmul(
                                pas[h][:, lo - QCW * c : lo - QCW * c + n],
                                v_sb[:, b * SKT + kt, h * VTC : (h + 1) * VTC],
                                exps[h][:, offs[kt] + lo - P * kt :
                                        offs[kt] + lo - P * kt + n],
                                start=(kt == 0),
                                stop=(kt == kt_hi - 1),
                            )
                    for h in range(HPC):
                        hp = h * hd
                        recip = spool.tile([1, QCW], F32, tag="recip",
                                           name=f"recip{b}_{c}_{h}")
                        nc.vector.reciprocal_approx_fast(
                            out=recip, in_=pas[h][hd : hd + 1, :]
                        )
                        rbc = spool.tile([hd, QCW], F32, tag="rbc",
                                         name=f"rbc{b}_{c}_{h}")
                        nc.gpsimd.partition_broadcast(rbc, recip[0:1, :])
                        nc.vector.tensor_tensor(
                            out=att_sb[hp : hp + hd, QCW * c : QCW * (c + 1)],
                            in0=pas[h][0:hd, :],
                            in1=rbc,
                            op=mybir.AluOpType.mult,
                        )

                # -- D: output projection for batch b (column-tile pairs) --
                for m in range(cfg.D // P):
                    for c in range(NQC):
                        po = psD.tile([P, QCW], F32, tag="po", name=f"po{b}_{m}_{c}")
                        PH = P // 2
                        for half in range(2):
                            nc.tensor.matmul(
                                po[half * PH : (half + 1) * PH, :],
                                wo_sb[:, m * P + half * PH : m * P + (half + 1) * PH],
                                att_sb[:, QCW * c : QCW * (c + 1)],
                                start=True,
                                stop=True,
                                skip_group_check=True,
                            )
                        o_sb = opool.tile([P, QCW], F32, tag="o", name=f"o{b}_{m}_{c}")
                        if (m * NQC + c) % 4 == 3:
                            nc.scalar.copy(out=o_sb, in_=po)
                        else:
                            nc.vector.tensor_copy(out=o_sb, in_=po)
                        nc.sync.dma_start(
                            out=yt[
                                m * P : (m + 1) * P, b * S + QCW * c : b * S + QCW * (c + 1)
                            ],
                            in_=o_sb,
                        )


def build_model(cfg: Cfg) -> bass.Bass:
    nc = bacc.Bacc(
        "TRN2", target_bir_lowering=False, debug=False, enable_asserts=False
    )
    io = {
        "xt": nc.dram_tensor("xt", [cfg.D, cfg.ROWS], F16, kind="ExternalInput").ap(),
        "wq": nc.dram_tensor("wq", [cfg.D, cfg.COLS], F16, kind="ExternalInput").ap(),
        "wk": nc.dram_tensor("wk", [cfg.D, cfg.COLS], F16, kind="ExternalInput").ap(),
        "wv": nc.dram_tensor("wv", [cfg.D, cfg.COLS], F16, kind="ExternalInput").ap(),
        "wo": nc.dram_tensor("wo", [cfg.COLS, cfg.D], F16, kind="ExternalInput").ap(),
        "bq": nc.dram_tensor("bq", [cfg.COLS], F32, kind="ExternalInput").ap(),
        "bk": nc.dram_tensor("bk", [cfg.COLS], F32, kind="ExternalInput").ap(),
        "bv": nc.dram_tensor("bv", [cfg.COLS], F32, kind="ExternalInput").ap(),
        "yt": nc.dram_tensor("yt", [cfg.D, cfg.ROWS], F32, kind="ExternalOutput").ap(),
    }
    with tile.TileContext(nc) as tc:
        emit_attention(tc, io, cfg)
    nc.finalize()
    return nc


def shard_inputs(cfg: Cfg, x, Wq, bq, Wk, bk, Wv, bv, Wo, bo):
    """Full inputs -> per-core in_maps (host side)."""
    scale = 1.0 / np.sqrt(np.float32(cfg.hd))
    xt = np.ascontiguousarray(
        np.asarray(x, dtype=np.float32).reshape(cfg.ROWS, cfg.D).T
    ).astype(np.float16)
    in_maps = []
    for c in range(cfg.NCORES):
        sl = slice(cfg.COLS * c, cfg.COLS * (c + 1))
        in_maps.append(
            {
                "xt": xt,
                "wq": np.ascontiguousarray(np.asarray(Wq)[:, sl] * scale).astype(np.float16),
                "bq": np.ascontiguousarray(np.asarray(bq)[sl] * scale),
                "wk": np.ascontiguousarray(np.asarray(Wk)[:, sl]).astype(np.float16),
                "bk": np.ascontiguousarray(np.asarray(bk)[sl]),
                "wv": np.ascontiguousarray(np.asarray(Wv)[:, sl]).astype(np.float16),
                "bv": np.ascontiguousarray(np.asarray(bv)[sl]),
                "wo": np.ascontiguousarray(np.asarray(Wo)[sl, :]).astype(np.float16),
            }
        )
    return in_maps


def unshard_output(cfg: Cfg, per_core_yt, bo):
    acc = per_core_yt[0].astype(np.float32)
    for yt_c in per_core_yt[1:]:
        acc = acc + yt_c
    y = acc.T + np.asarray(bo, dtype=np.float32)
    return np.ascontiguousarray(y.reshape(cfg.B, cfg.S, cfg.D)).astype(np.float32)


_MODEL = None


def _get_model(cfg: Cfg):
    global _MODEL
    if _MODEL is None:
        _MODEL = build_model(cfg)
    return _MODEL


def kernel(x, Wq, bq, Wk, bk, Wv, bv, Wo, bo, _trace=False):
    from concourse.bass_utils import run_bass_kernel_spmd

    cfg = Cfg()
    in_maps = shard_inputs(cfg, x, Wq, bq, Wk, bk, Wv, bv, Wo, bo)
    nc = _get_model(cfg)
    res = run_bass_kernel_spmd(
        nc, in_maps, core_ids=list(range(cfg.NCORES)), trace=_trace
    )
    y = unshard_output(cfg, [r["yt"] for r in res.results], bo)
    if _trace:
        return y, res
    return y


# revision 15
# speedup vs baseline: 1.2135x; 1.2135x over previous
"""Trainium2 Bass kernel for a causal multi-head attention block.

Reference computation (per nn_Attend):
    q = (x @ Wq + bq), k = (x @ Wk + bk), v = (x @ Wv + bv)   (per-head split)
    att = softmax(causal(q k^T / sqrt(hd)))
    y = (att v) @ Wo + bo

Sharding: tensor-parallel over heads across 8 NeuronCores. Core c gets
heads [2c, 2c+1]: column shards of Wq/Wk/Wv (+bias shards) and the matching
row shard of Wo. Every core computes a full-size partial output
yt_c = (att_out_c @ Wo_c)^T; the host sums the 8 partials, adds bo, and
transposes back.

On-chip layout is "transposed": activations live as [feature, token] so that
every matmul's contraction dim sits on SBUF partitions:
  QT/KT/VT = W^T @ x^T  (lhsT = W in natural [in,out] layout, rhs = x^T)
  scoresT[k, q] = (K^T)^T-slice @ Q^T      (contraction over head_dim)
  att_outT[hd, q] = sum_kt V[kt]^T-as-lhsT @ expT[kt]  (contraction over keys)
  yT[out, q] = Wo-slice-as-lhsT @ att_outT (contraction over per-core head dims)
V is needed in natural [token, hd] layout as lhsT; it is produced by PE
transposes of VT with an extra all-ones column so that each att_outT
accumulation also yields the softmax row-sums for free. Softmax is unmasked
exp (scores are bounded, no max subtraction needed); the causal mask is
"compute only the valid triangle" + a 0/1 upper-triangular mask multiply on
diagonal 128x128 blocks. Normalization divides att_outT columns by the
broadcast row-sums before the output projection.

All matmuls run in float16 (full PE rate, fast weight load).
"""

import os
from contextlib import ExitStack
from dataclasses import dataclass

import numpy as np

import concourse.bass as bass
import concourse.tile as tile
from concourse import bacc, mybir
from concourse.masks import make_identity

F32 = mybir.dt.float32
F32R = mybir.dt.float32r
F16 = mybir.dt.float16
P = 128  # SBUF partitions


@dataclass(frozen=True)
class Cfg:
    B: int = 2
    S: int = 2048
    D: int = 1024
    H: int = 16
    NCORES: int = 8
    RC: int = 512        # row chunk for QKV projection streaming
    PW: int = 1024       # psum piece width for scoresT (2 banks)

    @property
    def HPC(self):  # heads per core
        return self.H // self.NCORES

    @property
    def hd(self):  # head dim
        return self.D // self.H

    @property
    def COLS(self):  # per-core projection output columns
        return self.HPC * self.hd

    @property
    def KT(self):  # contraction tiles for projections
        return self.D // P

    @property
    def ROWS(self):
        return self.B * self.S

    @property
    def SKT(self):  # key tiles per batch
        return self.S // P

    @property
    def QCW(self):  # q chunk width for att_out accumulation
        return min(512, self.S)

    @property
    def NQC(self):
        return self.S // self.QCW

    @property
    def KPC(self):  # key tiles per q-chunk
        return self.QCW // P

    @property
    def VTC(self):  # v-tile columns per head (head_dim + ones column)
        return self.hd + 1


def _exp_offsets(cfg: Cfg):
    """Free-dim offsets of each key-tile's strip in the exp buffer."""
    offs, total = [], 0
    for kt in range(cfg.SKT):
        offs.append(total)
        total += cfg.S - P * kt
    return offs, total


def emit_attention(tc: tile.TileContext, io: dict, cfg: Cfg):
    nc = tc.nc
    xt, wq, wk, wv, wo = io["xt"], io["wq"], io["wk"], io["wv"], io["wo"]
    bq, bk, bv, yt = io["bq"], io["bk"], io["bv"], io["yt"]

    COLS, KT, RC, ROWS = cfg.COLS, cfg.KT, cfg.RC, cfg.ROWS
    S, SKT, B, hd, HPC = cfg.S, cfg.SKT, cfg.B, cfg.hd, cfg.HPC
    QCW, NQC, KPC, VTC, PW = cfg.QCW, cfg.NQC, cfg.KPC, cfg.VTC, cfg.PW
    NRCB = S // RC  # projection row-chunks per batch
    HC = COLS // 2  # projection column-tile half
    PH = P // 2     # out-projection column-tile half
    offs, expw = _exp_offsets(cfg)

    with ExitStack() as ctx:
        wpool = ctx.enter_context(tc.tile_pool(name="wpool", bufs=1))
        qkpool = ctx.enter_context(tc.tile_pool(name="qkpool", bufs=1))
        vpool = ctx.enter_context(tc.tile_pool(name="vpool", bufs=1))
        spool = ctx.enter_context(tc.tile_pool(name="spool", bufs=2))
        tpool = ctx.enter_context(tc.tile_pool(name="tpool", bufs=4))
        expool = ctx.enter_context(tc.tile_pool(name="expool", bufs=1))
        apool = ctx.enter_context(tc.tile_pool(name="apool", bufs=2))
        opool = ctx.enter_context(tc.tile_pool(name="opool", bufs=2))

        # ---- constants ----
        wq_sb = wpool.tile([P, KT, COLS], F16, tag="wq")
        wk_sb = wpool.tile([P, KT, COLS], F16, tag="wk")
        wv_sb = wpool.tile([P, KT, COLS], F16, tag="wv")
        wo_sb = wpool.tile([COLS, cfg.D], F16, tag="wo")
        bq_sb = wpool.tile([COLS, 1], F32, tag="bq")
        bk_sb = wpool.tile([COLS, 1], F32, tag="bk")
        bv_sb = wpool.tile([COLS, 1], F32, tag="bv")
        ident = wpool.tile([P, P], F16, tag="ident")

        for w_dram, w_sb in ((wq, wq_sb), (wk, wk_sb), (wv, wv_sb)):
            nc.sync.dma_start(
                out=w_sb, in_=w_dram.rearrange("(kt p) c -> p kt c", p=P)
            )
        nc.sync.dma_start(out=wo_sb, in_=wo)
        for b_dram, b_sb in ((bq, bq_sb), (bk, bk_sb), (bv, bv_sb)):
            nc.sync.dma_start(out=b_sb, in_=b_dram.rearrange("(c one) -> c one", one=1))
        make_identity(nc, ident)

        # per-batch activation buffers
        qt = [qkpool.tile([COLS, S], F16, tag=f"qt{b}", name=f"qt{b}") for b in range(B)]
        kts = [qkpool.tile([COLS, S], F16, tag=f"kt{b}", name=f"kt{b}") for b in range(B)]
        vts = [qkpool.tile([COLS, S], F16, tag=f"vt{b}", name=f"vt{b}") for b in range(B)]
        vs = [vpool.tile([P, SKT, HPC * VTC], F16, tag=f"v{b}", name=f"v{b}") for b in range(B)]
        atts = [apool.tile([COLS, S], F16, tag=f"att{b}", name=f"att{b}") for b in range(B)]
        for b in range(B):
            nc.vector.memset(vs[b][:, :, hd :: VTC], 1.0)

        with tc.tile_pool(name="psB", bufs=1, space="PSUM") as psB:

            # ---------- emitter helpers ----------
            def emit_proj_rc(b, rci):
                xt_t = xpool.tile([P, KT, RC], F16, tag="xt", name=f"xt{b}_{rci}")
                g0 = b * S + rci * RC
                nc.sync.dma_start(
                    out=xt_t,
                    in_=xt[:, g0 : g0 + RC].rearrange("(kt p) n -> p kt n", p=P),
                )
                for wi, (w_sb, b_sb, dest) in enumerate((
                    (wq_sb, bq_sb, qt[b]),
                    (wk_sb, bk_sb, kts[b]),
                    (wv_sb, bv_sb, vts[b]),
                )):
                    ps = psA.tile([COLS, RC], F32, tag="proj", name=f"pj{b}_{rci}_{wi}")
                    for kt in range(KT):
                        for half in range(2):
                            nc.tensor.matmul(
                                ps[half * HC : (half + 1) * HC, :],
                                w_sb[:, kt, half * HC : (half + 1) * HC],
                                xt_t[:, kt, :],
                                start=(kt == 0),
                                stop=(kt == KT - 1),
                                skip_group_check=True,
                            )
                    nc.vector.tensor_scalar_add(
                        dest[:, rci * RC : (rci + 1) * RC], ps, b_sb
                    )

            def emit_vtrans(b, t):
                pst = psT.tile([P, COLS], F16, tag="vtr", name=f"vtr{b}_{t}")
                nc.tensor.matmul(
                    pst,
                    vts[b][:, t * P : (t + 1) * P],
                    ident[:COLS, :COLS],
                    is_transpose=True,
                )
                nc.vector.tensor_copy(
                    out=vs[b][:, t, :].rearrange("p (h c) -> p h c", h=HPC)[:, :, 0:hd],
                    in_=pst.rearrange("p (h c) -> p h c", h=HPC),
                )

            def emit_B_strip(b, kt, exps):
                w = S - P * kt
                off = offs[kt]
                for p0 in range(0, w, PW):
                    pw_ = min(PW, w - p0)
                    pieces = [
                        psB.tile([P, PW], F32, tag=f"sc{h}", name=f"sc{b}_{kt}_{p0}_{h}")
                        for h in range(HPC)
                    ]
                    for s0 in range(0, pw_, 512):
                        sw = min(512, pw_ - s0)
                        for h in range(HPC):
                            hp = h * hd
                            nc.tensor.matmul(
                                pieces[h][:, s0 : s0 + sw],
                                kts[b][hp : hp + hd, P * kt : P * (kt + 1)],
                                qt[b][hp : hp + hd, P * kt + p0 + s0 : P * kt + p0 + s0 + sw],
                                start=True,
                                stop=True,
                            )
                    for h in range(HPC):
                        nc.scalar.activation(
                            out=exps[h][:, off + p0 : off + p0 + pw_],
                            in_=pieces[h][:, 0:pw_],
                            func=mybir.ActivationFunctionType.Exp,
                        )
                for h in range(HPC):
                    # causal mask on the diagonal block: keep q >= k
                    nc.gpsimd.affine_select(
                        out=exps[h][:, off : off + P],
                        in_=exps[h][:, off : off + P],
                        compare_op=mybir.AluOpType.is_ge,
                        fill=0.0,
                        base=0,
                        pattern=[[1, P]],
                        channel_multiplier=-1,
                    )

            def emit_C(b, exps):
                att_sb = atts[b]
                for c in range(NQC):
                    kt_hi = min(SKT, KPC * (c + 1))
                    pas = [
                        psC.tile([VTC, QCW], F32, tag="acc", name=f"acc{b}_{c}_{h}")
                        for h in range(HPC)
                    ]
                    for kt in range(kt_hi):
                        lo = max(QCW * c, P * kt)
                        n = QCW * (c + 1) - lo
                        if n <= 0:
                            continue
                        for h in range(HPC):
                            nc.tensor.matmul(
                                pas[h][:, lo - QCW * c : lo - QCW * c + n],
                                vs[b][:, kt, h * VTC : (h + 1) * VTC],
                                exps[h][:, offs[kt] + lo - P * kt :
                                        offs[kt] + lo - P * kt + n],
                                start=(kt == 0),
                                stop=(kt == kt_hi - 1),
                            )
                    for h in range(HPC):
                        hp = h * hd
                        # early PSUM release: copy to SBUF, normalize from there
                        tmp = tpool.tile([VTC, QCW], F32, tag="tmp",
                                         name=f"tmp{b}_{c}_{h}")
                        nc.vector.tensor_copy(out=tmp, in_=pas[h])
                        recip = spool.tile([1, QCW], F32, tag="recip",
                                           name=f"recip{b}_{c}_{h}")
                        nc.vector.reciprocal_approx_fast(
                            out=recip, in_=tmp[hd : hd + 1, :]
                        )
                        rbc = spool.tile([hd, QCW], F32, tag="rbc",
                                         name=f"rbc{b}_{c}_{h}")
                        nc.gpsimd.partition_broadcast(rbc, recip[0:1, :])
                        nc.vector.tensor_tensor(
                            out=att_sb[hp : hp + hd, QCW * c : QCW * (c + 1)],
                            in0=tmp[0:hd, :],
                            in1=rbc,
                            op=mybir.AluOpType.mult,
                        )

            o_tiles = {}

            def emit_D_chunk(b, m, c, engine_pick):
                att_sb = atts[b]
                if c == 0:
                    o_tiles[(b, m)] = opool.tile([P, S], F32, tag="o",
                                                 name=f"o{b}_{m}")
                o_sb = o_tiles[(b, m)]
                po = psD.tile([P, QCW], F32, tag="po", name=f"po{b}_{m}_{c}")
                for half in range(2):
                    nc.tensor.matmul(
                        po[half * PH : (half + 1) * PH, :],
                        wo_sb[:, m * P + half * PH : m * P + (half + 1) * PH],
                        att_sb[:, QCW * c : QCW * (c + 1)],
                        start=True,
                        stop=True,
                        skip_group_check=True,
                    )
                dst = o_sb[:, QCW * c : QCW * (c + 1)]
                if engine_pick % 4 == 3:
                    nc.scalar.copy(out=dst, in_=po)
                else:
                    nc.vector.tensor_copy(out=dst, in_=po)
                if c == NQC - 1:
                    nc.sync.dma_start(
                        out=yt[m * P : (m + 1) * P, b * S : (b + 1) * S],
                        in_=o_sb,
                    )

            # ---------- emission schedule ----------
            with (
                tc.tile_pool(name="xpool", bufs=2) as xpool,
                tc.tile_pool(name="psA", bufs=2, space="PSUM") as psA,
                tc.tile_pool(name="psT", bufs=2, space="PSUM") as psT,
            ):
                # window 1: batch-0 projections
                for rci in range(NRCB):
                    emit_proj_rc(0, rci)

                # window 1.5: batch-0 scores || batch-1 projections + V transposes
                exps0 = [
                    expool.tile([P, expw], F16, tag=f"exp{h}", name=f"exp0_{h}")
                    for h in range(HPC)
                ]
                aux = (
                    [("proj", 1, r) for r in range(NRCB)]
                    + [("vt", 0, t) for t in range(SKT)]
                    + [("vt", 1, t) for t in range(SKT)]
                )
                for kt in range(SKT):
                    emit_B_strip(0, kt, exps0)
                    lo = len(aux) * kt // SKT
                    hi = len(aux) * (kt + 1) // SKT
                    for kind, b_, i_ in aux[lo:hi]:
                        if kind == "proj":
                            emit_proj_rc(b_, i_)
                        else:
                            emit_vtrans(b_, i_)

            with (
                tc.tile_pool(name="psC", bufs=2, space="PSUM") as psC,
                tc.tile_pool(name="psD", bufs=2, space="PSUM") as psD,
            ):
                # window 2: batch-0 attention output
                emit_C(0, exps0)

                # window 3: batch-1 scores || batch-0 out-projection
                exps1 = [
                    expool.tile([P, expw], F16, tag=f"exp{h}", name=f"exp1_{h}")
                    for h in range(HPC)
                ]
                d0 = [(m, c) for m in range(cfg.D // P) for c in range(NQC)]
                for kt in range(SKT):
                    emit_B_strip(1, kt, exps1)
                    lo = len(d0) * kt // SKT
                    hi = len(d0) * (kt + 1) // SKT
                    for i, (m, c) in enumerate(d0[lo:hi]):
                        emit_D_chunk(0, m, c, lo + i)

                # window 4: batch-1 attention output + out-projection
                emit_C(1, exps1)
                for i, (m, c) in enumerate(d0):
                    emit_D_chunk(1, m, c, i)


def build_model(cfg: Cfg) -> bass.Bass:
    nc = bacc.Bacc(
        "TRN2", target_bir_lowering=False, debug=False, enable_asserts=False
    )
    io = {
        "xt": nc.dram_tensor("xt", [cfg.D, cfg.ROWS], F16, kind="ExternalInput").ap(),
        "wq": nc.dram_tensor("wq", [cfg.D, cfg.COLS], F16, kind="ExternalInput").ap(),
        "wk": nc.dram_tensor("wk", [cfg.D, cfg.COLS], F16, kind="ExternalInput").ap(),
        "wv": nc.dram_tensor("wv", [cfg.D, cfg.COLS], F16, kind="ExternalInput").ap(),
        "wo": nc.dram_tensor("wo", [cfg.COLS, cfg.D], F16, kind="ExternalInput").ap(),
        "bq": nc.dram_tensor("bq", [cfg.COLS], F32, kind="ExternalInput").ap(),
        "bk": nc.dram_tensor("bk", [cfg.COLS], F32, kind="ExternalInput").ap(),
        "bv": nc.dram_tensor("bv", [cfg.COLS], F32, kind="ExternalInput").ap(),
        "yt": nc.dram_tensor("yt", [cfg.D, cfg.ROWS], F32, kind="ExternalOutput").ap(),
    }
    with tile.TileContext(nc) as tc:
        emit_attention(tc, io, cfg)
    nc.finalize()
    return nc


def shard_inputs(cfg: Cfg, x, Wq, bq, Wk, bk, Wv, bv, Wo, bo):
    """Full inputs -> per-core in_maps (host side)."""
    scale = 1.0 / np.sqrt(np.float32(cfg.hd))
    xt = np.ascontiguousarray(
        np.asarray(x, dtype=np.float32).reshape(cfg.ROWS, cfg.D).T
    ).astype(np.float16)
    in_maps = []
    for c in range(cfg.NCORES):
        sl = slice(cfg.COLS * c, cfg.COLS * (c + 1))
        in_maps.append(
            {
                "xt": xt,
                "wq": np.ascontiguousarray(np.asarray(Wq)[:, sl] * scale).astype(np.float16),
                "bq": np.ascontiguousarray(np.asarray(bq)[sl] * scale),
                "wk": np.ascontiguousarray(np.asarray(Wk)[:, sl]).astype(np.float16),
                "bk": np.ascontiguousarray(np.asarray(bk)[sl]),
                "wv": np.ascontiguousarray(np.asarray(Wv)[:, sl]).astype(np.float16),
                "bv": np.ascontiguousarray(np.asarray(bv)[sl]),
                "wo": np.ascontiguousarray(np.asarray(Wo)[sl, :]).astype(np.float16),
            }
        )
    return in_maps


def unshard_output(cfg: Cfg, per_core_yt, bo):
    acc = per_core_yt[0].astype(np.float32)
    for yt_c in per_core_yt[1:]:
        acc = acc + yt_c
    y = acc.T + np.asarray(bo, dtype=np.float32)
    return np.ascontiguousarray(y.reshape(cfg.B, cfg.S, cfg.D)).astype(np.float32)


_MODEL = None


def _get_model(cfg: Cfg):
    global _MODEL
    if _MODEL is None:
        _MODEL = build_model(cfg)
    return _MODEL


def kernel(x, Wq, bq, Wk, bk, Wv, bv, Wo, bo, _trace=False):
    from concourse.bass_utils import run_bass_kernel_spmd

    cfg = Cfg()
    in_maps = shard_inputs(cfg, x, Wq, bq, Wk, bk, Wv, bv, Wo, bo)
    nc = _get_model(cfg)
    res = run_bass_kernel_spmd(
        nc, in_maps, core_ids=list(range(cfg.NCORES)), trace=_trace
    )
    y = unshard_output(cfg, [r["yt"] for r in res.results], bo)
    if _trace:
        return y, res
    return y


# revision 17
# speedup vs baseline: 1.2398x; 1.0217x over previous
"""Trainium2 Bass kernel for a causal multi-head attention block.

Reference computation (per nn_Attend):
    q = (x @ Wq + bq), k = (x @ Wk + bk), v = (x @ Wv + bv)   (per-head split)
    att = softmax(causal(q k^T / sqrt(hd)))
    y = (att v) @ Wo + bo

Sharding: tensor-parallel over heads across 8 NeuronCores. Core c gets
heads [2c, 2c+1]: column shards of Wq/Wk/Wv (+bias shards) and the matching
row shard of Wo. Every core computes a full-size partial output
yt_c = (att_out_c @ Wo_c)^T; the host sums the 8 partials, adds bo, and
transposes back.

On-chip layout is "transposed": activations live as [feature, token] so that
every matmul's contraction dim sits on SBUF partitions:
  QT/KT/VT = W^T @ x^T  (lhsT = W in natural [in,out] layout, rhs = x^T)
  scoresT[k, q] = (K^T)^T-slice @ Q^T      (contraction over head_dim)
  att_outT[hd, q] = sum_kt V[kt]^T-as-lhsT @ expT[kt]  (contraction over keys)
  yT[out, q] = Wo-slice-as-lhsT @ att_outT (contraction over per-core head dims)
V is needed in natural [token, hd] layout as lhsT; it is produced by PE
transposes of VT with an extra all-ones column so that each att_outT
accumulation also yields the softmax row-sums for free. Softmax is unmasked
exp (scores are bounded, no max subtraction needed); the causal mask is
"compute only the valid triangle" + a 0/1 upper-triangular mask multiply on
diagonal 128x128 blocks. Normalization divides att_outT columns by the
broadcast row-sums before the output projection.

All matmuls run in float16 (full PE rate, fast weight load).
"""

import os
from contextlib import ExitStack
from dataclasses import dataclass

import numpy as np

import concourse.bass as bass
import concourse.tile as tile
from concourse import bacc, mybir
from concourse.masks import make_identity

F32 = mybir.dt.float32
F32R = mybir.dt.float32r
F16 = mybir.dt.float16
P = 128  # SBUF partitions


@dataclass(frozen=True)
class Cfg:
    B: int = 2
    S: int = 2048
    D: int = 1024
    H: int = 16
    NCORES: int = 8
    RC: int = 512        # row chunk for QKV projection streaming
    PW: int = 1024       # psum piece width for scoresT (2 banks)

    @property
    def HPC(self):  # heads per core
        return self.H // self.NCORES

    @property
    def hd(self):  # head dim
        return self.D // self.H

    @property
    def COLS(self):  # per-core projection output columns
        return self.HPC * self.hd

    @property
    def KT(self):  # contraction tiles for projections
        return self.D // P

    @property
    def ROWS(self):
        return self.B * self.S

    @property
    def SKT(self):  # key tiles per batch
        return self.S // P

    @property
    def QCW(self):  # q chunk width for att_out accumulation
        return min(512, self.S)

    @property
    def NQC(self):
        return self.S // self.QCW

    @property
    def KPC(self):  # key tiles per q-chunk
        return self.QCW // P

    @property
    def VTC(self):  # v-tile columns per head (head_dim + ones column)
        return self.hd + 1


def _exp_offsets(cfg: Cfg):
    """Free-dim offsets of each key-tile's strip in the exp buffer."""
    offs, total = [], 0
    for kt in range(cfg.SKT):
        offs.append(total)
        total += cfg.S - P * kt
    return offs, total


def emit_attention(tc: tile.TileContext, io: dict, cfg: Cfg):
    nc = tc.nc
    xt, wq, wk, wv, wo = io["xt"], io["wq"], io["wk"], io["wv"], io["wo"]
    bq, bk, bv, yt = io["bq"], io["bk"], io["bv"], io["yt"]

    COLS, KT, RC, ROWS = cfg.COLS, cfg.KT, cfg.RC, cfg.ROWS
    S, SKT, B, hd, HPC = cfg.S, cfg.SKT, cfg.B, cfg.hd, cfg.HPC
    QCW, NQC, KPC, VTC, PW = cfg.QCW, cfg.NQC, cfg.KPC, cfg.VTC, cfg.PW
    NRCB = S // RC  # projection row-chunks per batch
    HC = COLS // 2  # projection column-tile half
    PH = P // 2     # out-projection column-tile half
    offs, expw = _exp_offsets(cfg)

    with ExitStack() as ctx:
        wpool = ctx.enter_context(tc.tile_pool(name="wpool", bufs=1))
        qkpool = ctx.enter_context(tc.tile_pool(name="qkpool", bufs=1))
        vpool = ctx.enter_context(tc.tile_pool(name="vpool", bufs=1))
        spool = ctx.enter_context(tc.tile_pool(name="spool", bufs=2))
        tpool = ctx.enter_context(tc.tile_pool(name="tpool", bufs=4))
        expool = ctx.enter_context(tc.tile_pool(name="expool", bufs=1))
        apool = ctx.enter_context(tc.tile_pool(name="apool", bufs=2))
        opool = ctx.enter_context(tc.tile_pool(name="opool", bufs=2))

        # ---- constants ----
        wq_sb = wpool.tile([P, KT, COLS], F16, tag="wq")
        wk_sb = wpool.tile([P, KT, COLS], F16, tag="wk")
        wv_sb = wpool.tile([P, KT, COLS], F16, tag="wv")
        wo_sb = wpool.tile([COLS, cfg.D], F16, tag="wo")
        bq_sb = wpool.tile([COLS, 1], F32, tag="bq")
        bk_sb = wpool.tile([COLS, 1], F32, tag="bk")
        bv_sb = wpool.tile([COLS, 1], F32, tag="bv")
        ident = wpool.tile([P, P], F16, tag="ident")

        for w_dram, w_sb in ((wq, wq_sb), (wk, wk_sb), (wv, wv_sb)):
            nc.sync.dma_start(
                out=w_sb, in_=w_dram.rearrange("(kt p) c -> p kt c", p=P)
            )
        nc.sync.dma_start(out=wo_sb, in_=wo)
        for b_dram, b_sb in ((bq, bq_sb), (bk, bk_sb), (bv, bv_sb)):
            nc.sync.dma_start(out=b_sb, in_=b_dram.rearrange("(c one) -> c one", one=1))
        make_identity(nc, ident)

        # per-batch activation buffers
        qt = [qkpool.tile([COLS, S], F16, tag=f"qt{b}", name=f"qt{b}") for b in range(B)]
        kts = [qkpool.tile([COLS, S], F16, tag=f"kt{b}", name=f"kt{b}") for b in range(B)]
        vts = [qkpool.tile([COLS, S], F16, tag=f"vt{b}", name=f"vt{b}") for b in range(B)]
        vs = [vpool.tile([P, SKT, HPC * VTC], F16, tag=f"v{b}", name=f"v{b}") for b in range(B)]
        atts = [apool.tile([COLS, S], F16, tag=f"att{b}", name=f"att{b}") for b in range(B)]
        for b in range(B):
            nc.vector.memset(vs[b][:, :, hd :: VTC], 1.0)

        with tc.tile_pool(name="psB", bufs=1, space="PSUM") as psB:

            # ---------- emitter helpers ----------
            def emit_proj_rc(b, rci):
                xt_t = xpool.tile([P, KT, RC], F16, tag="xt", name=f"xt{b}_{rci}")
                g0 = b * S + rci * RC
                nc.sync.dma_start(
                    out=xt_t,
                    in_=xt[:, g0 : g0 + RC].rearrange("(kt p) n -> p kt n", p=P),
                )
                for wi, (w_sb, b_sb, dest) in enumerate((
                    (wq_sb, bq_sb, qt[b]),
                    (wk_sb, bk_sb, kts[b]),
                    (wv_sb, bv_sb, vts[b]),
                )):
                    ps = psA.tile([COLS, RC], F32, tag="proj", name=f"pj{b}_{rci}_{wi}")
                    for kt in range(KT):
                        for half in range(2):
                            nc.tensor.matmul(
                                ps[half * HC : (half + 1) * HC, :],
                                w_sb[:, kt, half * HC : (half + 1) * HC],
                                xt_t[:, kt, :],
                                start=(kt == 0),
                                stop=(kt == KT - 1),
                                skip_group_check=True,
                            )
                    nc.vector.tensor_scalar_add(
                        dest[:, rci * RC : (rci + 1) * RC], ps, b_sb
                    )

            def emit_vtrans(b, t):
                pst = psT.tile([P, COLS], F16, tag="vtr", name=f"vtr{b}_{t}")
                nc.tensor.matmul(
                    pst,
                    vts[b][:, t * P : (t + 1) * P],
                    ident[:COLS, :COLS],
                    is_transpose=True,
                )
                nc.vector.tensor_copy(
                    out=vs[b][:, t, :].rearrange("p (h c) -> p h c", h=HPC)[:, :, 0:hd],
                    in_=pst.rearrange("p (h c) -> p h c", h=HPC),
                )

            def emit_B_strip(b, kt, exps):
                w = S - P * kt
                off = offs[kt]
                for p0 in range(0, w, PW):
                    pw_ = min(PW, w - p0)
                    pieces = [
                        psB.tile([P, PW], F32, tag=f"sc{h}", name=f"sc{b}_{kt}_{p0}_{h}")
                        for h in range(HPC)
                    ]
                    for s0 in range(0, pw_, 512):
                        sw = min(512, pw_ - s0)
                        for h in range(HPC):
                            hp = h * hd
                            nc.tensor.matmul(
                                pieces[h][:, s0 : s0 + sw],
                                kts[b][hp : hp + hd, P * kt : P * (kt + 1)],
                                qt[b][hp : hp + hd, P * kt + p0 + s0 : P * kt + p0 + s0 + sw],
                                start=True,
                                stop=True,
                            )
                    for h in range(HPC):
                        nc.scalar.activation(
                            out=exps[h][:, off + p0 : off + p0 + pw_],
                            in_=pieces[h][:, 0:pw_],
                            func=mybir.ActivationFunctionType.Exp,
                        )
                for h in range(HPC):
                    # causal mask on the diagonal block: keep q >= k
                    nc.gpsimd.affine_select(
                        out=exps[h][:, off : off + P],
                        in_=exps[h][:, off : off + P],
                        compare_op=mybir.AluOpType.is_ge,
                        fill=0.0,
                        base=0,
                        pattern=[[1, P]],
                        channel_multiplier=-1,
                    )

            def emit_C(b, exps, with_D=False):
                att_sb = atts[b]
                for c in range(NQC):
                    kt_hi = min(SKT, KPC * (c + 1))
                    pas = [
                        psC.tile([VTC, QCW], F32, tag="acc", name=f"acc{b}_{c}_{h}")
                        for h in range(HPC)
                    ]
                    for kt in range(kt_hi):
                        lo = max(QCW * c, P * kt)
                        n = QCW * (c + 1) - lo
                        if n <= 0:
                            continue
                        for h in range(HPC):
                            nc.tensor.matmul(
                                pas[h][:, lo - QCW * c : lo - QCW * c + n],
                                vs[b][:, kt, h * VTC : (h + 1) * VTC],
                                exps[h][:, offs[kt] + lo - P * kt :
                                        offs[kt] + lo - P * kt + n],
                                start=(kt == 0),
                                stop=(kt == kt_hi - 1),
                            )
                    for h in range(HPC):
                        hp = h * hd
                        # early PSUM release: copy to SBUF, normalize from there
                        tmp = tpool.tile([VTC, QCW], F32, tag="tmp",
                                         name=f"tmp{b}_{c}_{h}")
                        nc.vector.tensor_copy(out=tmp, in_=pas[h])
                        recip = spool.tile([1, QCW], F32, tag="recip",
                                           name=f"recip{b}_{c}_{h}")
                        nc.vector.reciprocal_approx_fast(
                            out=recip, in_=tmp[hd : hd + 1, :]
                        )
                        rbc = spool.tile([hd, QCW], F32, tag="rbc",
                                         name=f"rbc{b}_{c}_{h}")
                        nc.gpsimd.partition_broadcast(rbc, recip[0:1, :])
                        nc.vector.tensor_tensor(
                            out=att_sb[hp : hp + hd, QCW * c : QCW * (c + 1)],
                            in0=tmp[0:hd, :],
                            in1=rbc,
                            op=mybir.AluOpType.mult,
                        )
                    if with_D:
                        for m in range(cfg.D // P):
                            emit_Dc_chunk(b, m, c, m + c)

            o_tiles = {}

            def emit_Dc_chunk(b, m, c, engine_pick):
                att_sb = atts[b]
                po = psD.tile([P, QCW], F32, tag="po", name=f"pc{b}_{m}_{c}")
                for half in range(2):
                    nc.tensor.matmul(
                        po[half * PH : (half + 1) * PH, :],
                        wo_sb[:, m * P + half * PH : m * P + (half + 1) * PH],
                        att_sb[:, QCW * c : QCW * (c + 1)],
                        start=True,
                        stop=True,
                        skip_group_check=True,
                    )
                oc = opool.tile([P, QCW], F32, tag="oc", name=f"oc{b}_{m}_{c}", bufs=3)
                if engine_pick % 4 == 3:
                    nc.scalar.copy(out=oc, in_=po)
                else:
                    nc.vector.tensor_copy(out=oc, in_=po)
                nc.sync.dma_start(
                    out=yt[m * P : (m + 1) * P,
                           b * S + QCW * c : b * S + QCW * (c + 1)],
                    in_=oc,
                )

            def emit_D_chunk(b, m, c, engine_pick):
                att_sb = atts[b]
                if c == 0:
                    o_tiles[(b, m)] = opool.tile([P, S], F32, tag="o",
                                                 name=f"o{b}_{m}")
                o_sb = o_tiles[(b, m)]
                po = psD.tile([P, QCW], F32, tag="po", name=f"po{b}_{m}_{c}")
                for half in range(2):
                    nc.tensor.matmul(
                        po[half * PH : (half + 1) * PH, :],
                        wo_sb[:, m * P + half * PH : m * P + (half + 1) * PH],
                        att_sb[:, QCW * c : QCW * (c + 1)],
                        start=True,
                        stop=True,
                        skip_group_check=True,
                    )
                dst = o_sb[:, QCW * c : QCW * (c + 1)]
                if engine_pick % 4 == 3:
                    nc.scalar.copy(out=dst, in_=po)
                else:
                    nc.vector.tensor_copy(out=dst, in_=po)
                if c == NQC - 1:
                    nc.sync.dma_start(
                        out=yt[m * P : (m + 1) * P, b * S : (b + 1) * S],
                        in_=o_sb,
                    )

            # ---------- emission schedule ----------
            with (
                tc.tile_pool(name="xpool", bufs=2) as xpool,
                tc.tile_pool(name="psA", bufs=2, space="PSUM") as psA,
                tc.tile_pool(name="psT", bufs=2, space="PSUM") as psT,
            ):
                # window 1: batch-0 projections
                for rci in range(NRCB):
                    emit_proj_rc(0, rci)

                # window 1.5: batch-0 scores || batch-1 projections + V transposes
                exps0 = [
                    expool.tile([P, expw], F16, tag=f"exp{h}", name=f"exp0_{h}")
                    for h in range(HPC)
                ]
                aux = (
                    [("proj", 1, r) for r in range(NRCB)]
                    + [("vt", 0, t) for t in range(SKT)]
                    + [("vt", 1, t) for t in range(SKT)]
                )
                for kt in range(SKT):
                    emit_B_strip(0, kt, exps0)
                    lo = len(aux) * kt // SKT
                    hi = len(aux) * (kt + 1) // SKT
                    for kind, b_, i_ in aux[lo:hi]:
                        if kind == "proj":
                            emit_proj_rc(b_, i_)
                        else:
                            emit_vtrans(b_, i_)

            with (
                tc.tile_pool(name="psC", bufs=2, space="PSUM") as psC,
                tc.tile_pool(name="psD", bufs=2, space="PSUM") as psD,
            ):
                # window 2: batch-0 attention output
                emit_C(0, exps0)

                # window 3: batch-1 scores || batch-0 out-projection
                exps1 = [
                    expool.tile([P, expw], F16, tag=f"exp{h}", name=f"exp1_{h}")
                    for h in range(HPC)
                ]
                d0 = [(m, c) for m in range(cfg.D // P) for c in range(NQC)]
                for kt in range(SKT):
                    emit_B_strip(1, kt, exps1)
                    lo = len(d0) * kt // SKT
                    hi = len(d0) * (kt + 1) // SKT
                    for i, (m, c) in enumerate(d0[lo:hi]):
                        emit_D_chunk(0, m, c, lo + i)

                # window 4: batch-1 attention output + out-projection,
                # pipelined per q-chunk
                emit_C(1, exps1, with_D=True)


def build_model(cfg: Cfg) -> bass.Bass:
    nc = bacc.Bacc(
        "TRN2", target_bir_lowering=False, debug=False, enable_asserts=False
    )
    io = {
        "xt": nc.dram_tensor("xt", [cfg.D, cfg.ROWS], F16, kind="ExternalInput").ap(),
        "wq": nc.dram_tensor("wq", [cfg.D, cfg.COLS], F16, kind="ExternalInput").ap(),
        "wk": nc.dram_tensor("wk", [cfg.D, cfg.COLS], F16, kind="ExternalInput").ap(),
        "wv": nc.dram_tensor("wv", [cfg.D, cfg.COLS], F16, kind="ExternalInput").ap(),
        "wo": nc.dram_tensor("wo", [cfg.COLS, cfg.D], F16, kind="ExternalInput").ap(),
        "bq": nc.dram_tensor("bq", [cfg.COLS], F32, kind="ExternalInput").ap(),
        "bk": nc.dram_tensor("bk", [cfg.COLS], F32, kind="ExternalInput").ap(),
        "bv": nc.dram_tensor("bv", [cfg.COLS], F32, kind="ExternalInput").ap(),
        "yt": nc.dram_tensor("yt", [cfg.D, cfg.ROWS], F32, kind="ExternalOutput").ap(),
    }
    with tile.TileContext(nc) as tc:
        emit_attention(tc, io, cfg)
    nc.finalize()
    return nc


def shard_inputs(cfg: Cfg, x, Wq, bq, Wk, bk, Wv, bv, Wo, bo):
    """Full inputs -> per-core in_maps (host side)."""
    scale = 1.0 / np.sqrt(np.float32(cfg.hd))
    xt = np.ascontiguousarray(
        np.asarray(x, dtype=np.float32).reshape(cfg.ROWS, cfg.D).T
    ).astype(np.float16)
    in_maps = []
    for c in range(cfg.NCORES):
        sl = slice(cfg.COLS * c, cfg.COLS * (c + 1))
        in_maps.append(
            {
                "xt": xt,
                "wq": np.ascontiguousarray(np.asarray(Wq)[:, sl] * scale).astype(np.float16),
                "bq": np.ascontiguousarray(np.asarray(bq)[sl] * scale),
                "wk": np.ascontiguousarray(np.asarray(Wk)[:, sl]).astype(np.float16),
                "bk": np.ascontiguousarray(np.asarray(bk)[sl]),
                "wv": np.ascontiguousarray(np.asarray(Wv)[:, sl]).astype(np.float16),
                "bv": np.ascontiguousarray(np.asarray(bv)[sl]),
                "wo": np.ascontiguousarray(np.asarray(Wo)[sl, :]).astype(np.float16),
            }
        )
    return in_maps


def unshard_output(cfg: Cfg, per_core_yt, bo):
    acc = per_core_yt[0].astype(np.float32)
    for yt_c in per_core_yt[1:]:
        acc = acc + yt_c
    y = acc.T + np.asarray(bo, dtype=np.float32)
    return np.ascontiguousarray(y.reshape(cfg.B, cfg.S, cfg.D)).astype(np.float32)


_MODEL = None


def _get_model(cfg: Cfg):
    global _MODEL
    if _MODEL is None:
        _MODEL = build_model(cfg)
    return _MODEL


def kernel(x, Wq, bq, Wk, bk, Wv, bv, Wo, bo, _trace=False):
    from concourse.bass_utils import run_bass_kernel_spmd

    cfg = Cfg()
    in_maps = shard_inputs(cfg, x, Wq, bq, Wk, bk, Wv, bv, Wo, bo)
    nc = _get_model(cfg)
    res = run_bass_kernel_spmd(
        nc, in_maps, core_ids=list(range(cfg.NCORES)), trace=_trace
    )
    y = unshard_output(cfg, [r["yt"] for r in res.results], bo)
    if _trace:
        return y, res
    return y


# revision 18
# speedup vs baseline: 1.3000x; 1.0486x over previous
"""Trainium2 Bass kernel for a causal multi-head attention block.

Reference computation (per nn_Attend):
    q = (x @ Wq + bq), k = (x @ Wk + bk), v = (x @ Wv + bv)   (per-head split)
    att = softmax(causal(q k^T / sqrt(hd)))
    y = (att v) @ Wo + bo

Sharding: tensor-parallel over heads across 8 NeuronCores. Core c gets
heads [2c, 2c+1]: column shards of Wq/Wk/Wv (+bias shards) and the matching
row shard of Wo. Every core computes a full-size partial output
yt_c = (att_out_c @ Wo_c)^T; the host sums the 8 partials, adds bo, and
transposes back.

On-chip layout is "transposed": activations live as [feature, token] so that
every matmul's contraction dim sits on SBUF partitions:
  QT/KT/VT = W^T @ x^T  (lhsT = W in natural [in,out] layout, rhs = x^T)
  scoresT[k, q] = (K^T)^T-slice @ Q^T      (contraction over head_dim)
  att_outT[hd, q] = sum_kt V[kt]^T-as-lhsT @ expT[kt]  (contraction over keys)
  yT[out, q] = Wo-slice-as-lhsT @ att_outT (contraction over per-core head dims)
V is needed in natural [token, hd] layout as lhsT; it is produced by PE
transposes of VT with an extra all-ones column so that each att_outT
accumulation also yields the softmax row-sums for free. Softmax is unmasked
exp (scores are bounded, no max subtraction needed); the causal mask is
"compute only the valid triangle" + a 0/1 upper-triangular mask multiply on
diagonal 128x128 blocks. Normalization divides att_outT columns by the
broadcast row-sums before the output projection.

All matmuls run in float16 (full PE rate, fast weight load).
"""

import os
from contextlib import ExitStack
from dataclasses import dataclass

import numpy as np

import concourse.bass as bass
import concourse.tile as tile
from concourse import bacc, mybir
from concourse.masks import make_identity

F32 = mybir.dt.float32
F32R = mybir.dt.float32r
F16 = mybir.dt.float16
P = 128  # SBUF partitions


@dataclass(frozen=True)
class Cfg:
    B: int = 2
    S: int = 2048
    D: int = 1024
    H: int = 16
    NCORES: int = 8
    RC: int = 512        # row chunk for QKV projection streaming
    PW: int = 1024       # psum piece width for scoresT (2 banks)

    @property
    def HPC(self):  # heads per core
        return self.H // self.NCORES

    @property
    def hd(self):  # head dim
        return self.D // self.H

    @property
    def COLS(self):  # per-core projection output columns
        return self.HPC * self.hd

    @property
    def KT(self):  # contraction tiles for projections
        return self.D // P

    @property
    def ROWS(self):
        return self.B * self.S

    @property
    def SKT(self):  # key tiles per batch
        return self.S // P

    @property
    def QCW(self):  # q chunk width for att_out accumulation
        return min(512, self.S)

    @property
    def NQC(self):
        return self.S // self.QCW

    @property
    def KPC(self):  # key tiles per q-chunk
        return self.QCW // P

    @property
    def VTC(self):  # v-tile columns per head (head_dim + ones column)
        return self.hd + 1


def _exp_offsets(cfg: Cfg):
    """Free-dim offsets of each key-tile's strip in the exp buffer."""
    offs, total = [], 0
    for kt in range(cfg.SKT):
        offs.append(total)
        total += cfg.S - P * kt
    return offs, total


def emit_attention(tc: tile.TileContext, io: dict, cfg: Cfg):
    nc = tc.nc
    xt, wq, wk, wv, wo = io["xt"], io["wq"], io["wk"], io["wv"], io["wo"]
    bq, bk, bv, yt = io["bq"], io["bk"], io["bv"], io["yt"]

    COLS, KT, RC, ROWS = cfg.COLS, cfg.KT, cfg.RC, cfg.ROWS
    S, SKT, B, hd, HPC = cfg.S, cfg.SKT, cfg.B, cfg.hd, cfg.HPC
    QCW, NQC, KPC, VTC, PW = cfg.QCW, cfg.NQC, cfg.KPC, cfg.VTC, cfg.PW
    NRCB = S // RC  # projection row-chunks per batch
    HC = COLS // 2  # projection column-tile half
    PH = P // 2     # out-projection column-tile half
    offs, expw = _exp_offsets(cfg)

    with ExitStack() as ctx:
        wpool = ctx.enter_context(tc.tile_pool(name="wpool", bufs=1))
        qkpool = ctx.enter_context(tc.tile_pool(name="qkpool", bufs=1))
        vpool = ctx.enter_context(tc.tile_pool(name="vpool", bufs=1))
        spool = ctx.enter_context(tc.tile_pool(name="spool", bufs=2))
        tpool = ctx.enter_context(tc.tile_pool(name="tpool", bufs=4))
        expool = ctx.enter_context(tc.tile_pool(name="expool", bufs=1))
        apool = ctx.enter_context(tc.tile_pool(name="apool", bufs=2))
        opool = ctx.enter_context(tc.tile_pool(name="opool", bufs=2))

        # ---- constants ----
        wq_sb = wpool.tile([P, KT, COLS], F16, tag="wq")
        wk_sb = wpool.tile([P, KT, COLS], F16, tag="wk")
        wv_sb = wpool.tile([P, KT, COLS], F16, tag="wv")
        wo_sb = wpool.tile([COLS, cfg.D], F16, tag="wo")
        bq_sb = wpool.tile([COLS, 1], F32, tag="bq")
        bk_sb = wpool.tile([COLS, 1], F32, tag="bk")
        bv_sb = wpool.tile([COLS, 1], F32, tag="bv")
        ident = wpool.tile([P, P], F16, tag="ident")

        for w_dram, w_sb in ((wq, wq_sb), (wk, wk_sb), (wv, wv_sb)):
            nc.sync.dma_start(
                out=w_sb, in_=w_dram.rearrange("(kt p) c -> p kt c", p=P)
            )
        nc.sync.dma_start(out=wo_sb, in_=wo)
        for b_dram, b_sb in ((bq, bq_sb), (bk, bk_sb), (bv, bv_sb)):
            nc.sync.dma_start(out=b_sb, in_=b_dram.rearrange("(c one) -> c one", one=1))
        make_identity(nc, ident)

        # per-batch activation buffers
        qt = [qkpool.tile([COLS, S], F16, tag=f"qt{b}", name=f"qt{b}") for b in range(B)]
        kts = [qkpool.tile([COLS, S], F16, tag=f"kt{b}", name=f"kt{b}") for b in range(B)]
        vts = [qkpool.tile([COLS, S], F16, tag=f"vt{b}", name=f"vt{b}") for b in range(B)]
        vs = [vpool.tile([P, SKT, HPC * VTC], F16, tag=f"v{b}", name=f"v{b}") for b in range(B)]
        atts = [apool.tile([COLS, S], F16, tag=f"att{b}", name=f"att{b}") for b in range(B)]
        for b in range(B):
            nc.vector.memset(vs[b][:, :, hd :: VTC], 1.0)

        with tc.tile_pool(name="psB", bufs=1, space="PSUM") as psB:

            # ---------- emitter helpers ----------
            def emit_proj_rc(b, rci):
                xt_t = xpool.tile([P, KT, RC], F16, tag="xt", name=f"xt{b}_{rci}")
                g0 = b * S + rci * RC
                nc.sync.dma_start(
                    out=xt_t,
                    in_=xt[:, g0 : g0 + RC].rearrange("(kt p) n -> p kt n", p=P),
                )
                for wi, (w_sb, b_sb, dest) in enumerate((
                    (wq_sb, bq_sb, qt[b]),
                    (wk_sb, bk_sb, kts[b]),
                    (wv_sb, bv_sb, vts[b]),
                )):
                    ps = psA.tile([COLS, RC], F32, tag="proj", name=f"pj{b}_{rci}_{wi}")
                    for kt in range(KT):
                        for half in range(2):
                            nc.tensor.matmul(
                                ps[half * HC : (half + 1) * HC, :],
                                w_sb[:, kt, half * HC : (half + 1) * HC],
                                xt_t[:, kt, :],
                                start=(kt == 0),
                                stop=(kt == KT - 1),
                                skip_group_check=True,
                            )
                    nc.vector.tensor_scalar_add(
                        dest[:, rci * RC : (rci + 1) * RC], ps, b_sb
                    )

            def emit_vtrans(b, t):
                pst = psT.tile([P, COLS], F16, tag="vtr", name=f"vtr{b}_{t}")
                nc.tensor.matmul(
                    pst,
                    vts[b][:, t * P : (t + 1) * P],
                    ident[:COLS, :COLS],
                    is_transpose=True,
                )
                nc.vector.tensor_copy(
                    out=vs[b][:, t, :].rearrange("p (h c) -> p h c", h=HPC)[:, :, 0:hd],
                    in_=pst.rearrange("p (h c) -> p h c", h=HPC),
                )

            def emit_B_strip(b, kt, exps):
                w = S - P * kt
                off = offs[kt]
                for p0 in range(0, w, PW):
                    pw_ = min(PW, w - p0)
                    pieces = [
                        psB.tile([P, PW], F32, tag=f"sc{h}", name=f"sc{b}_{kt}_{p0}_{h}")
                        for h in range(HPC)
                    ]
                    for s0 in range(0, pw_, 512):
                        sw = min(512, pw_ - s0)
                        for h in range(HPC):
                            hp = h * hd
                            for kh in range(2):  # k-row column-tile halves
                                nc.tensor.matmul(
                                    pieces[h][kh * PH : (kh + 1) * PH, s0 : s0 + sw],
                                    kts[b][hp : hp + hd,
                                          P * kt + kh * PH : P * kt + (kh + 1) * PH],
                                    qt[b][hp : hp + hd,
                                          P * kt + p0 + s0 : P * kt + p0 + s0 + sw],
                                    start=True,
                                    stop=True,
                                    skip_group_check=True,
                                )
                    for h in range(HPC):
                        nc.scalar.activation(
                            out=exps[h][:, off + p0 : off + p0 + pw_],
                            in_=pieces[h][:, 0:pw_],
                            func=mybir.ActivationFunctionType.Exp,
                        )
                for h in range(HPC):
                    # causal mask on the diagonal block: keep q >= k
                    nc.gpsimd.affine_select(
                        out=exps[h][:, off : off + P],
                        in_=exps[h][:, off : off + P],
                        compare_op=mybir.AluOpType.is_ge,
                        fill=0.0,
                        base=0,
                        pattern=[[1, P]],
                        channel_multiplier=-1,
                    )

            def emit_C(b, exps, with_D=False):
                att_sb = atts[b]
                for c in range(NQC):
                    kt_hi = min(SKT, KPC * (c + 1))
                    pas = [
                        psC.tile([VTC, QCW], F32, tag="acc", name=f"acc{b}_{c}_{h}")
                        for h in range(HPC)
                    ]
                    for kt in range(kt_hi):
                        lo = max(QCW * c, P * kt)
                        n = QCW * (c + 1) - lo
                        if n <= 0:
                            continue
                        for h in range(HPC):
                            nc.tensor.matmul(
                                pas[h][:, lo - QCW * c : lo - QCW * c + n],
                                vs[b][:, kt, h * VTC : (h + 1) * VTC],
                                exps[h][:, offs[kt] + lo - P * kt :
                                        offs[kt] + lo - P * kt + n],
                                start=(kt == 0),
                                stop=(kt == kt_hi - 1),
                            )
                    for h in range(HPC):
                        hp = h * hd
                        # early PSUM release: copy to SBUF, normalize from there
                        tmp = tpool.tile([VTC, QCW], F32, tag="tmp",
                                         name=f"tmp{b}_{c}_{h}")
                        nc.vector.tensor_copy(out=tmp, in_=pas[h])
                        recip = spool.tile([1, QCW], F32, tag="recip",
                                           name=f"recip{b}_{c}_{h}")
                        nc.vector.reciprocal_approx_fast(
                            out=recip, in_=tmp[hd : hd + 1, :]
                        )
                        rbc = spool.tile([hd, QCW], F32, tag="rbc",
                                         name=f"rbc{b}_{c}_{h}")
                        nc.gpsimd.partition_broadcast(rbc, recip[0:1, :])
                        nc.vector.tensor_tensor(
                            out=att_sb[hp : hp + hd, QCW * c : QCW * (c + 1)],
                            in0=tmp[0:hd, :],
                            in1=rbc,
                            op=mybir.AluOpType.mult,
                        )
                    if with_D:
                        for m in range(cfg.D // P):
                            emit_Dc_chunk(b, m, c, m + c)

            o_tiles = {}

            def emit_Dc_chunk(b, m, c, engine_pick):
                att_sb = atts[b]
                po = psD.tile([P, QCW], F32, tag="po", name=f"pc{b}_{m}_{c}")
                for half in range(2):
                    nc.tensor.matmul(
                        po[half * PH : (half + 1) * PH, :],
                        wo_sb[:, m * P + half * PH : m * P + (half + 1) * PH],
                        att_sb[:, QCW * c : QCW * (c + 1)],
                        start=True,
                        stop=True,
                        skip_group_check=True,
                    )
                oc = opool.tile([P, QCW], F32, tag="oc", name=f"oc{b}_{m}_{c}", bufs=3)
                if engine_pick % 4 == 3:
                    nc.scalar.copy(out=oc, in_=po)
                else:
                    nc.vector.tensor_copy(out=oc, in_=po)
                nc.sync.dma_start(
                    out=yt[m * P : (m + 1) * P,
                           b * S + QCW * c : b * S + QCW * (c + 1)],
                    in_=oc,
                )

            def emit_D_chunk(b, m, c, engine_pick):
                att_sb = atts[b]
                if c == 0:
                    o_tiles[(b, m)] = opool.tile([P, S], F32, tag="o",
                                                 name=f"o{b}_{m}")
                o_sb = o_tiles[(b, m)]
                po = psD.tile([P, QCW], F32, tag="po", name=f"po{b}_{m}_{c}")
                for half in range(2):
                    nc.tensor.matmul(
                        po[half * PH : (half + 1) * PH, :],
                        wo_sb[:, m * P + half * PH : m * P + (half + 1) * PH],
                        att_sb[:, QCW * c : QCW * (c + 1)],
                        start=True,
                        stop=True,
                        skip_group_check=True,
                    )
                dst = o_sb[:, QCW * c : QCW * (c + 1)]
                if engine_pick % 4 == 3:
                    nc.scalar.copy(out=dst, in_=po)
                else:
                    nc.vector.tensor_copy(out=dst, in_=po)
                if c == NQC - 1:
                    nc.sync.dma_start(
                        out=yt[m * P : (m + 1) * P, b * S : (b + 1) * S],
                        in_=o_sb,
                    )

            # ---------- emission schedule ----------
            with (
                tc.tile_pool(name="xpool", bufs=2) as xpool,
                tc.tile_pool(name="psA", bufs=2, space="PSUM") as psA,
                tc.tile_pool(name="psT", bufs=2, space="PSUM") as psT,
            ):
                # window 1: batch-0 projections
                for rci in range(NRCB):
                    emit_proj_rc(0, rci)

                # window 1.5: batch-0 scores || batch-1 projections + V transposes
                exps0 = [
                    expool.tile([P, expw], F16, tag=f"exp{h}", name=f"exp0_{h}")
                    for h in range(HPC)
                ]
                aux = (
                    [("proj", 1, r) for r in range(NRCB)]
                    + [("vt", 0, t) for t in range(SKT)]
                    + [("vt", 1, t) for t in range(SKT)]
                )
                for kt in range(SKT):
                    emit_B_strip(0, kt, exps0)
                    lo = len(aux) * kt // SKT
                    hi = len(aux) * (kt + 1) // SKT
                    for kind, b_, i_ in aux[lo:hi]:
                        if kind == "proj":
                            emit_proj_rc(b_, i_)
                        else:
                            emit_vtrans(b_, i_)

            with (
                tc.tile_pool(name="psC", bufs=2, space="PSUM") as psC,
                tc.tile_pool(name="psD", bufs=2, space="PSUM") as psD,
            ):
                # window 2: batch-0 attention output
                emit_C(0, exps0)

                # window 3: batch-1 scores || batch-0 out-projection
                exps1 = [
                    expool.tile([P, expw], F16, tag=f"exp{h}", name=f"exp1_{h}")
                    for h in range(HPC)
                ]
                d0 = [(m, c) for m in range(cfg.D // P) for c in range(NQC)]
                for kt in range(SKT):
                    emit_B_strip(1, kt, exps1)
                    lo = len(d0) * kt // SKT
                    hi = len(d0) * (kt + 1) // SKT
                    for i, (m, c) in enumerate(d0[lo:hi]):
                        emit_D_chunk(0, m, c, lo + i)

                # window 4: batch-1 attention output + out-projection,
                # pipelined per q-chunk
                emit_C(1, exps1, with_D=True)


def build_model(cfg: Cfg) -> bass.Bass:
    nc = bacc.Bacc(
        "TRN2", target_bir_lowering=False, debug=False, enable_asserts=False
    )
    io = {
        "xt": nc.dram_tensor("xt", [cfg.D, cfg.ROWS], F16, kind="ExternalInput").ap(),
        "wq": nc.dram_tensor("wq", [cfg.D, cfg.COLS], F16, kind="ExternalInput").ap(),
        "wk": nc.dram_tensor("wk", [cfg.D, cfg.COLS], F16, kind="ExternalInput").ap(),
        "wv": nc.dram_tensor("wv", [cfg.D, cfg.COLS], F16, kind="ExternalInput").ap(),
        "wo": nc.dram_tensor("wo", [cfg.COLS, cfg.D], F16, kind="ExternalInput").ap(),
        "bq": nc.dram_tensor("bq", [cfg.COLS], F32, kind="ExternalInput").ap(),
        "bk": nc.dram_tensor("bk", [cfg.COLS], F32, kind="ExternalInput").ap(),
        "bv": nc.dram_tensor("bv", [cfg.COLS], F32, kind="ExternalInput").ap(),
        "yt": nc.dram_tensor("yt", [cfg.D, cfg.ROWS], F32, kind="ExternalOutput").ap(),
    }
    with tile.TileContext(nc) as tc:
        emit_attention(tc, io, cfg)
    nc.finalize()
    return nc


def shard_inputs(cfg: Cfg, x, Wq, bq, Wk, bk, Wv, bv, Wo, bo):
    """Full inputs -> per-core in_maps (host side)."""
    scale = 1.0 / np.sqrt(np.float32(cfg.hd))
    xt = np.ascontiguousarray(
        np.asarray(x, dtype=np.float32).reshape(cfg.ROWS, cfg.D).T
    ).astype(np.float16)
    in_maps = []
    for c in range(cfg.NCORES):
        sl = slice(cfg.COLS * c, cfg.COLS * (c + 1))
        in_maps.append(
            {
                "xt": xt,
                "wq": np.ascontiguousarray(np.asarray(Wq)[:, sl] * scale).astype(np.float16),
                "bq": np.ascontiguousarray(np.asarray(bq)[sl] * scale),
                "wk": np.ascontiguousarray(np.asarray(Wk)[:, sl]).astype(np.float16),
                "bk": np.ascontiguousarray(np.asarray(bk)[sl]),
                "wv": np.ascontiguousarray(np.asarray(Wv)[:, sl]).astype(np.float16),
                "bv": np.ascontiguousarray(np.asarray(bv)[sl]),
                "wo": np.ascontiguousarray(np.asarray(Wo)[sl, :]).astype(np.float16),
            }
        )
    return in_maps


def unshard_output(cfg: Cfg, per_core_yt, bo):
    acc = per_core_yt[0].astype(np.float32)
    for yt_c in per_core_yt[1:]:
        acc = acc + yt_c
    y = acc.T + np.asarray(bo, dtype=np.float32)
    return np.ascontiguousarray(y.reshape(cfg.B, cfg.S, cfg.D)).astype(np.float32)


_MODEL = None


def _get_model(cfg: Cfg):
    global _MODEL
    if _MODEL is None:
        _MODEL = build_model(cfg)
    return _MODEL


def kernel(x, Wq, bq, Wk, bk, Wv, bv, Wo, bo, _trace=False):
    from concourse.bass_utils import run_bass_kernel_spmd

    cfg = Cfg()
    in_maps = shard_inputs(cfg, x, Wq, bq, Wk, bk, Wv, bv, Wo, bo)
    nc = _get_model(cfg)
    res = run_bass_kernel_spmd(
        nc, in_maps, core_ids=list(range(cfg.NCORES)), trace=_trace
    )
    y = unshard_output(cfg, [r["yt"] for r in res.results], bo)
    if _trace:
        return y, res
    return y
